# revision 30
# baseline (speedup 1.0000x reference)
"""Trainium2 Bass kernel for nn_CNNMIMOEncoder.

Model: 8x [3x3 conv (1 chan, image 2x4096) -> BatchNorm(train, global stats)
-> ELU], flatten -> Linear 8192->3072 -> split into (u, v), v L2-normalized
per sample.

Strategy (8 NeuronCores, pure data parallel over batch B=4096 -> 512/core):
- Host pre-transposes x to pixel-major "interleaved" layout g = 2*omega + c
  (omega = (nrx*32+ntx)*8 + k), zero-padded, in overlapping 128-row tiles
  advancing by 122 pixels (3-pixel halo each side).
- Conv = ONE TensorE matmul per tile with a host-built 128x122 banded
  stationary matrix (float32r -> full-rate fp32). Conv bias is dropped:
  a uniform bias cancels exactly through training-mode BatchNorm.
- BN statistics: per-op accum_out partial sums (S1 via the PSUM->SBUF
  evacuation on ScalarE, S2 = sum of squares via a VectorE STT), reduced
  on-chip, then a tiny 8-core AllReduce (2 floats) per block.
- BN+ELU fused: h' = min(exp(s*h+t) - 1, relu(s*h+t)) with rsqrt via
  reciprocal + sqrt + one Newton step.
- Linear: out[o, b] accumulated over 68 pixel-tiles in PSUM, W^T streamed
  from DRAM (host-permuted to pixel order), float32r full-rate. v-norm via
  Square + ones-matmul partition reduction, rsqrt, broadcast multiply.
"""

import os
import numpy as np

NB = 8          # conv blocks
NPIX = 8192     # 2 rows * 4096 interleaved pixels
TOUT = 122      # output pixels per conv tile
NT = 68         # ceil(8192/122)
NCORES = 8
BN_EPS = 1e-5
XROWS = 122 * (NT - 1) + 128  # 8302
NOUT = 3072
NV = 2048       # v columns (first), u columns = 1024 (last)

_CACHE = {}


def _build_band(conv_w: np.ndarray) -> np.ndarray:
    """[8,1,1,3,3] -> [8,128,3,128] banded stationary matrix variants.

    Tile u partition p <-> interleaved pixel g = 122u + p - 3 (g = 2j + r).
    Output local index m -> pixel 122u + m - 3 (full 128-row output; the 6
    halo rows come out wrong and are fixed by halo DMAs / masked from stats).
    Variant 0: tile 0 (left zero-pad: input rows 0..2 zeroed).
    Variant 1: interior tiles.
    Variant 2: tile 67 (right edge: rows >= 21 and cols >= 21 zeroed).
    Layout [q, variant, m] so one DMA loads [128, 3*128] per block.
    """
    S = np.zeros((NB, 128, 3, 128), np.float32)
    for i in range(NB):
        w = conv_w[i, 0, 0]  # [kh, kw]
        base = np.zeros((128, 128), np.float32)
        for m in range(128):
            r = (m + 1) % 2
            for dq in range(-3, 4):
                q = m + dq
                if not (0 <= q < 128):
                    continue
                rr = (r + dq) % 2
                kh = (rr - r) + 1
                kw = (dq - (rr - r)) // 2 + 1
                if 0 <= kh <= 2 and 0 <= kw <= 2 and (dq - (rr - r)) % 2 == 0:
                    base[q, m] = w[kh, kw]
        s0 = base.copy()
        s0[0:3, :] = 0.0
        s67 = base.copy()
        s67[21:, :] = 0.0
        s67[:, 21:] = 0.0
        S[i, :, 0, :] = s0
        S[i, :, 1, :] = base
        S[i, :, 2, :] = s67
    return S


def _build_nc(bcore: int, single: bool = False):
    import concourse.bass as bass
    import concourse.mybir as mybir
    import concourse.tile as tile
    from concourse import bacc
    from contextlib import ExitStack

    f32 = mybir.dt.float32
    f32r = mybir.dt.float32r
    AX = mybir.AxisListType
    OP = mybir.AluOpType
    AF = mybir.ActivationFunctionType
    B = bcore
    NTOT = float(NCORES * bcore * NPIX)  # BN element count

    nc = bacc.Bacc("TRN2", target_bir_lowering=False, debug=False,
                   num_devices=1 if single else NCORES)

    xt = nc.dram_tensor("xt", [XROWS, B], f32, kind="ExternalInput")
    wt = nc.dram_tensor("wt", [NT * 128, NOUT], f32, kind="ExternalInput")
    sband = nc.dram_tensor("sband", [NB, 128, 3 * 128], f32,
                           kind="ExternalInput")
    mask = nc.dram_tensor("mask", [128, 1], f32, kind="ExternalInput")
    bpar = nc.dram_tensor("bpar", [1, 2 * NB], f32, kind="ExternalInput")
    blin = nc.dram_tensor("blin", [1, NOUT], f32, kind="ExternalInput")
    outy = nc.dram_tensor("outy", [NOUT, B], f32, kind="ExternalOutput")
    cci = [nc.dram_tensor(f"cci{i}", [1, 2], f32) for i in range(NB)]
    cco = [nc.dram_tensor(f"cco{i}", [1, 2], f32, addr_space="Shared")
           for i in range(NB)]
    rg = [list(range(NCORES))]

    with tile.TileContext(nc) as tc, ExitStack() as ctx:
        sb = ctx.enter_context(tc.tile_pool(name="persist", bufs=1))
        Hb = sb.tile([128, NT * B], f32)       # the big h buffer
        H3 = Hb[:].rearrange("p (t b) -> p t b", t=NT)
        ones = sb.tile([128, max(512, B)], f32)
        msk = sb.tile([128, 1], f32)
        S1 = sb.tile([128, NT], f32)
        S2 = sb.tile([128, NT], f32)
        Sred = sb.tile([128, 2], f32)
        scr = sb.tile([1, 16], f32)            # scalar scratch
        strow = sb.tile([1, 2], f32)           # (s, t)
        stbc = sb.tile([128, 2], f32)          # broadcast (s, t)
        bp_sb = sb.tile([1, 2 * NB], f32)
        blin_sb = sb.tile([1, NOUT], f32)
        tot = sb.tile([1, 2], f32)

        nc.vector.memset(ones[:], 1.0)
        nc.sync.dma_start(msk[:], mask.ap())
        nc.sync.dma_start(bp_sb[:], bpar.ap())
        nc.sync.dma_start(blin_sb[:], blin.ap())

        # initial load: overlapping 128-row tiles from padded xt
        for u in range(NT):
            nc.sync.dma_start(H3[:, u, :].bitcast(f32r),
                              xt.ap()[122 * u:122 * u + 128, :].bitcast(f32r))

        with ExitStack() as cctx:
            sbp = cctx.enter_context(tc.tile_pool(name="sbandp", bufs=2))
            psc = cctx.enter_context(tc.tile_pool(name="psconv", bufs=3,
                                                  space="PSUM"))
            ps1 = cctx.enter_context(tc.tile_pool(name="pssmall", bufs=1,
                                                  space="PSUM"))
            sqp = cctx.enter_context(tc.tile_pool(name="sqp", bufs=2))
            ep = cctx.enter_context(tc.tile_pool(name="ep", bufs=3))
            rp = cctx.enter_context(tc.tile_pool(name="rp", bufs=3))

            for i in range(NB):
                sbi = sbp.tile([128, 3 * 128], f32)
                nc.sync.dma_start(sbi[:].bitcast(f32r),
                                  sband.ap()[i].bitcast(f32r))
                nc.vector.memset(S1[:], 0.0)
                nc.vector.memset(S2[:], 0.0)

                for u0 in range(0, NT, 2):
                    ps = psc.tile([128, 2 * B], f32)
                    for k in range(2):
                        u = u0 + k
                        var = 0 if u == 0 else (2 if u == NT - 1 else 1)
                        nc.tensor.matmul(
                            ps[:, k * B:(k + 1) * B],
                            sbi[:, var * 128:(var + 1) * 128].bitcast(f32r),
                            H3[:, u, :].bitcast(f32r),
                            start=True, stop=True)
                    # evacuate raw conv out in place (+ per-pair sum)
                    nc.scalar.activation(
                        Hb[:, u0 * B:(u0 + 2) * B].bitcast(f32r),
                        ps[:, :], AF.Copy,
                        accum_out=S1[:, u0 // 2:u0 // 2 + 1])
                    # sum of squares (psum * sbuf-raw)
                    sq = sqp.tile([128, 2 * B], f32)
                    nc.vector.scalar_tensor_tensor(
                        sq[:, :], ps[:, :], 1.0, Hb[:, u0 * B:(u0 + 2) * B],
                        op0=OP.mult, op1=OP.mult,
                        accum_out=S2[:, u0 // 2:u0 // 2 + 1])

                # halo duplication (raw) for next block's overlapped tiles
                nc.sync.dma_start(H3[0:3, 1:NT, :].bitcast(f32r),
                                  H3[122:125, 0:NT - 1, :].bitcast(f32r))
                nc.sync.dma_start(H3[125:128, 0:NT - 1, :].bitcast(f32r),
                                  H3[3:6, 1:NT, :].bitcast(f32r))

                # stats: reduce tiles, reduce partitions, all-reduce
                nc.vector.reduce_sum(Sred[:, 0:1], S1[:], axis=AX.X)
                nc.vector.reduce_sum(Sred[:, 1:2], S2[:], axis=AX.X)
                pst = ps1.tile([1, 2], f32)
                nc.tensor.matmul(pst[0:1, :], msk[:, 0:1], Sred[:, :],
                                 start=True, stop=True)
                psb_out = sbp.tile([1, 2], f32, tag="ccout")
                nc.scalar.activation(psb_out[0:1, :], pst[0:1, :], AF.Copy)
                nc.sync.dma_start(cci[i].ap(), psb_out[0:1, :])
                if single:
                    # timeline-model variant: skip the collective (its ~10us
                    # floor is added analytically); keep the DRAM round trip
                    nc.sync.dma_start(cco[i].ap(), cci[i].ap())
                else:
                    nc.gpsimd.collective_compute(
                        "AllReduce", OP.add,
                        ins=[cci[i].ap()], outs=[cco[i].ap()],
                        replica_groups=rg)
                nc.sync.dma_start(tot[0:1, :], cco[i].ap())

                # mean, var, s = g*rsqrt(var+eps), t = b - s*mean
                mn = scr[0:1, 0:1]
                ex2 = scr[0:1, 1:2]
                nvar = scr[0:1, 2:3]
                xve = scr[0:1, 3:4]
                y0 = scr[0:1, 4:5]
                y0s = scr[0:1, 5:6]
                aa = scr[0:1, 6:7]
                cc = scr[0:1, 7:8]
                y1 = scr[0:1, 8:9]
                smean = scr[0:1, 9:10]
                rxe = scr[0:1, 10:11]
                nc.vector.tensor_scalar(mn, tot[0:1, 0:1], 1.0 / NTOT, None,
                                        op0=OP.mult)
                nc.vector.tensor_scalar(ex2, tot[0:1, 1:2], 1.0 / NTOT, None,
                                        op0=OP.mult)
                # mean^2 - Ex2 = -var
                nc.vector.scalar_tensor_tensor(nvar, mn, mn, ex2,
                                               op0=OP.mult, op1=OP.subtract)
                # x = var + eps
                nc.vector.tensor_scalar(xve, nvar, -1.0, BN_EPS,
                                        op0=OP.mult, op1=OP.add)
                # y0 ~= rsqrt(x) = sqrt(1/x); Newton: y1 = y0*(1.5-0.5*x*y0^2)
                nc.vector.reciprocal(rxe, xve)
                nc.scalar.activation(y0, rxe, AF.Sqrt)
                nc.vector.scalar_tensor_tensor(y0s, y0, 1.0, y0,
                                               op0=OP.mult, op1=OP.mult)
                nc.vector.tensor_scalar(aa, y0s, xve, -0.5,
                                        op0=OP.mult, op1=OP.mult)
                nc.vector.tensor_scalar(cc, aa, 1.5, None, op0=OP.add)
                nc.vector.tensor_tensor(y1, y0, cc, op=OP.mult)
                # s = g * y1 ; t = b - s*mean
                nc.vector.tensor_scalar(strow[0:1, 0:1], y1,
                                        bp_sb[0:1, 2 * i:2 * i + 1], None,
                                        op0=OP.mult)
                nc.vector.scalar_tensor_tensor(
                    smean, strow[0:1, 0:1], mn, bp_sb[0:1, 2 * i + 1:2 * i + 2],
                    op0=OP.mult, op1=OP.subtract)
                nc.vector.tensor_scalar(strow[0:1, 1:2], smean, -1.0, None,
                                        op0=OP.mult)
                # broadcast (s,t) across partitions via ones-matmul
                psb = ps1.tile([128, 2], f32)
                nc.tensor.matmul(psb[:, :], ones[0:1, 0:128], strow[0:1, :],
                                 start=True, stop=True)
                nc.scalar.activation(stbc[:], psb[:, :], AF.Copy)

                # phase 2: h' = min(exp(y)-1, relu(y)), y = s*h + t
                CH = 2  # tiles per chunk
                for c in range(NT // CH):
                    hs = Hb[:, c * CH * B:(c + 1) * CH * B]
                    et = ep.tile([128, CH * B], f32)
                    rt = rp.tile([128, CH * B], f32)
                    nc.scalar.activation(et[:], hs, AF.Exp,
                                         bias=stbc[:, 1:2], scale=stbc[:, 0:1])
                    nc.scalar.activation(rt[:], hs, AF.Relu,
                                         bias=stbc[:, 1:2], scale=stbc[:, 0:1])
                    nc.vector.scalar_tensor_tensor(hs.bitcast(f32r), et[:],
                                                   -1.0, rt[:],
                                                   op0=OP.add, op1=OP.min)
                # (no re-zeroing needed: S0/S67 stationary variants zero out
                #  contributions from polluted pad regions)

        # ---- Linear: out[o, b] = W^T.T @ h + b_lin; v-normalize ----
        wtp = ctx.enter_context(tc.tile_pool(name="wtp", bufs=8))
        psl = ctx.enter_context(tc.tile_pool(name="pslin", bufs=2, space="PSUM"))
        pss = ctx.enter_context(tc.tile_pool(name="psss", bufs=1, space="PSUM"))
        sqp2 = ctx.enter_context(tc.tile_pool(name="sqp2", bufs=3))
        vs = ctx.enter_context(tc.tile_pool(name="vsb", bufs=1))
        nrm = sb.tile([1, B], f32)
        inv = sb.tile([1, B], f32)
        invbc = vs.tile([128, B], f32)
        pssum = pss.tile([1, B], f32)

        NOT = NOUT // 128  # 24 o-tiles
        NVT = NV // 128    # 16 v o-tiles
        for ot in range(NOT):
            ps = psl.tile([128, B], f32)
            nc.tensor.matmul(ps[:, :], blin_sb[0:1, 128 * ot:128 * (ot + 1)],
                             ones[0:1, 0:B], start=True, stop=False)
            for u in range(NT):
                wtt = wtp.tile([128, 128], f32)
                nc.sync.dma_start(
                    wtt[:, :].bitcast(f32r),
                    wt.ap()[128 * u:128 * (u + 1),
                            128 * ot:128 * (ot + 1)].bitcast(f32r))
                nc.tensor.matmul(ps[:, :], wtt[:, :].bitcast(f32r),
                                 H3[:, u, :].bitcast(f32r),
                                 start=False, stop=(u == NT - 1))
            uo = sqp2.tile([128, B], f32, tag="uout")
            nc.scalar.activation(uo[:], ps[:, :], AF.Copy)
            nc.sync.dma_start(outy.ap()[128 * ot:128 * (ot + 1), :], uo[:])
            if ot < NVT:
                sqt = sqp2.tile([128, B], f32)
                nc.scalar.activation(sqt[:], ps[:, :], AF.Square)
                nc.tensor.matmul(pssum[0:1, :], ones[0:128, 0:1], sqt[:, :],
                                 start=(ot == 0), stop=(ot == NVT - 1))

        # inv_norm = rsqrt(sum v^2) with one Newton step
        r0 = sb.tile([1, B], f32)
        rx = sb.tile([1, B], f32)
        t0 = sb.tile([1, B], f32)
        t1 = sb.tile([1, B], f32)
        nc.scalar.activation(nrm[0:1, :], pssum[0:1, :], AF.Copy)
        nc.vector.reciprocal(rx[0:1, :], nrm[0:1, :])
        nc.scalar.activation(r0[0:1, :], rx[0:1, :], AF.Sqrt)
        # Newton for rsqrt around x = sumsq: y1 = y0*(1.5 - 0.5*x*y0^2)
        nc.vector.scalar_tensor_tensor(t0[0:1, :], r0[0:1, :], 1.0, r0[0:1, :],
                                       op0=OP.mult, op1=OP.mult)
        nc.vector.tensor_tensor(t1[0:1, :], t0[0:1, :], nrm[0:1, :],
                                op=OP.mult)
        nc.vector.tensor_scalar(t0[0:1, :], t1[0:1, :], -0.5, 1.5,
                                op0=OP.mult, op1=OP.add)
        nc.vector.tensor_tensor(inv[0:1, :], r0[0:1, :], t0[0:1, :],
                                op=OP.mult)
        # broadcast inv across 128 partitions
        psb2 = psl.tile([128, B], f32, tag="ps")
        nc.tensor.matmul(psb2[:, :], ones[0:1, 0:128], inv[0:1, :],
                         start=True, stop=True)
        nc.scalar.activation(invbc[:], psb2[:, :], AF.Copy)
        # re-read unscaled v from DRAM, scale, write back
        for ot in range(NVT):
            vi = sqp2.tile([128, B], f32, tag="uout")
            nc.sync.dma_start(vi[:], outy.ap()[128 * ot:128 * (ot + 1), :])
            vo = sqp2.tile([128, B], f32)
            nc.vector.tensor_tensor(vo[:], vi[:], invbc[:], op=OP.mult)
            nc.sync.dma_start(outy.ap()[128 * ot:128 * (ot + 1), :], vo[:])

    nc.compile()
    return nc


def _prep_inputs(x, conv_w, bn_g, bn_b, W, b_lin, bcore):
    xr = np.ascontiguousarray(
        np.transpose(np.asarray(x, np.float32), (2, 3, 1, 4, 0))
    ).reshape(NPIX, -1)
    xt_full = np.zeros((XROWS, xr.shape[1]), np.float32)
    xt_full[3:3 + NPIX] = xr
    wt_perm = (np.asarray(W, np.float32).T.reshape(2, NPIX // 2, NOUT)
               .transpose(1, 0, 2).reshape(NPIX, NOUT))
    # overlapped 128-row tiles: row u*128+q <- wt_perm[122u+q-3] for q in
    # [3,125) only (halo rows zero so duplicated pixels aren't double-counted)
    wtp = np.zeros((NT * 128, NOUT), np.float32)
    for u in range(NT):
        lo = 122 * u
        hi = min(lo + TOUT, NPIX)
        wtp[u * 128 + 3:u * 128 + 3 + (hi - lo)] = wt_perm[lo:hi]
    sb = _build_band(np.asarray(conv_w, np.float32)).reshape(NB, 128, 3 * 128)
    mk = np.ones((128, 1), np.float32)
    mk[0:3] = 0.0
    mk[125:128] = 0.0
    bp = np.zeros((1, 2 * NB), np.float32)
    bp[0, 0::2] = np.asarray(bn_g, np.float32).reshape(-1)
    bp[0, 1::2] = np.asarray(bn_b, np.float32).reshape(-1)
    bl = np.asarray(b_lin, np.float32).reshape(1, NOUT)
    in_maps = []
    for c in range(NCORES):
        in_maps.append({
            "xt": np.ascontiguousarray(xt_full[:, c * bcore:(c + 1) * bcore]),
            "wt": wtp,
            "sband": sb,
            "mask": mk,
            "bpar": bp,
            "blin": bl,
        })
    return in_maps


def run(inputs: dict, trace: bool = False, bcore: int | None = None):
    """Build (cached), run on 8 cores, return (results, BassKernelResults)."""
    from concourse.bass_utils import run_bass_kernel_spmd

    x = inputs["x"]
    B = x.shape[0]
    if bcore is None:
        bcore = B // NCORES
    assert B == bcore * NCORES
    if bcore not in _CACHE:
        _CACHE[bcore] = _build_nc(bcore)
    nc = _CACHE[bcore]
    in_maps = _prep_inputs(x, inputs["conv_w"], inputs["bn_g"],
                           inputs["bn_b"], inputs["W"], inputs["b_lin"], bcore)
    res = run_bass_kernel_spmd(nc, in_maps, core_ids=list(range(NCORES)),
                               trace=trace)
    ys = [res.results[c]["outy"] for c in range(NCORES)]  # [3072, bcore] each
    y = np.concatenate(ys, axis=1).T  # [B, 3072]
    v = y[:, :NV].reshape(B, 8, 32, 4, 2)
    u = y[:, NV:].reshape(B, 8, 16, 4, 2)
    return (u, v), res


def kernel(**inputs) -> tuple:
    out, _ = run(inputs, trace=False)
    return out
